# revision 29
# baseline (speedup 1.0000x reference)
"""CrossAttention (B=2, N=M=2048, D=1024, H=16, DH=64) on 8 Trainium2 cores.

Sharding: data-parallel over batch x tensor-parallel over heads (Megatron).
Core c handles batch b=c//4 and heads [4*(c%4), 4*(c%4)+4).  Wq/Wk/Wv are
column-sharded, Wo row-sharded; each core emits a partial [N, D] output and
the host sums the 4 partials per batch (+ bo) at gather time.

Per-core kernel (v3, tuned against the TimelineSim cost model):
  - Projections produce Q^T / K^T (weights stationary, x^T / ctx^T moving)
    and V in natural [m, dh] layout with a ones column per head appended.
  - Scores S^T[keys, q] per (head, ktile) in fp32 PSUM, exp on ACT in
    [128, 1024] batches straight out of PSUM into bf16 SBUF tiles.
  - AV uses the "tall" form: stationary = P^T chunk [128 keys, 128 q],
    moving = V+ones [128 keys, 65] so each matmul streams only 65 columns;
    output [128 q, 64 dh | denom] accumulates over the 16 key tiles in a
    single PSUM bank (4 q-subtiles share the bank; one start/stop per bank).
  - Normalisation: reciprocal of the denominator column + broadcast
    tensor-tensor multiply on DVE (per-q scalars live on partitions).
  - O is transposed back to O^T via PE transpose-mode (identity matrix) so
    the output projection can consume it as the stationary operand.
  - Projection work that is off the startup critical path is emitted via an
    explicit per-(pair, head, kchunk) schedule inside the first 512-q block
    so the PE never head-blocks and the exp stream starts at ~14us.

Matmul inputs are bf16; accumulation, exp and normalisation are fp32.
"""

from contextlib import ExitStack

import ml_dtypes
import numpy as np

import concourse.bass as bass
import concourse.mybir as mybir
import concourse.tile as tile
from concourse import bacc
from concourse.bass_utils import run_bass_kernel_spmd
from concourse.masks import make_identity

B, N, M, D = 2, 2048, 2048, 1024
H, DH = 16, 64
SCALE = DH ** -0.5
NCORES = 8
CPB = 4              # cores per batch
HL = H // CPB        # 4 local heads per core
DL = HL * DH         # 256 local head dims
KC = D // 128        # 8 contraction chunks for the projections
PAIRS = HL // 2      # head pairs packed 2-per-128-partitions
KT = M // 128        # 16 key tiles
QHB = 512            # q block granule for the main loop
NQHB = N // QHB      # 4

f32 = mybir.dt.float32
bf16 = mybir.dt.bfloat16
i16 = mybir.dt.int16
EXP_A = float(2 ** 7 / np.log(2))         # Schraudolph exp in bf16:
EXP_B = float(127 * 2 ** 7 - 7.32)        # bitcast_bf16(round(A*s + B))
np_bf16 = ml_dtypes.bfloat16
EXP = mybir.ActivationFunctionType.Exp


def build():
    """Build the single SPMD Bass program (same NEFF for all 8 cores)."""
    nc = bacc.Bacc("TRN2", target_bir_lowering=False, debug=False)

    xT = nc.dram_tensor("xT", [D, N], bf16, kind="ExternalInput").ap()
    ctxT = nc.dram_tensor("ctxT", [D, M], bf16, kind="ExternalInput").ap()
    wq = nc.dram_tensor("wq", [D, DL], bf16, kind="ExternalInput").ap()
    wk = nc.dram_tensor("wk", [D, DL], bf16, kind="ExternalInput").ap()
    wv = nc.dram_tensor("wv", [D, DL], bf16, kind="ExternalInput").ap()
    wo = nc.dram_tensor("wo", [DL, D], bf16, kind="ExternalInput").ap()
    out = nc.dram_tensor("out", [N, D], bf16, kind="ExternalOutput").ap()

    with tile.TileContext(nc) as tc, ExitStack() as ctx:
        wpool = ctx.enter_context(tc.tile_pool(name="w", bufs=1))
        dpool = ctx.enter_context(tc.tile_pool(name="data", bufs=1))
        kqpool = ctx.enter_context(tc.tile_pool(name="kq", bufs=1))
        ots_pool = ctx.enter_context(tc.tile_pool(name="ots", bufs=1))
        pt_pool = ctx.enter_context(tc.tile_pool(name="pt", bufs=10))
        ob_pool = ctx.enter_context(tc.tile_pool(name="ob", bufs=2))
        nrm_pool = ctx.enter_context(tc.tile_pool(name="nrm", bufs=4))
        osb_pool = ctx.enter_context(tc.tile_pool(name="osb", bufs=4))
        # PSUM budget: st 4 banks + ot 2 + aux 2 (shared with transposes) = 8
        st_ps = ctx.enter_context(tc.tile_pool(name="st_ps", bufs=3, space="PSUM"))
        ot_ps = ctx.enter_context(tc.tile_pool(name="ot_ps", bufs=1, space="PSUM"))
        aux_ps = ctx.enter_context(tc.tile_pool(name="aux_ps", bufs=1, space="PSUM"))

        # ---- input DMAs: batched, in startup-criticality order ----
        def w_dma(nm, t):
            wt = wpool.tile([128, KC, DL], bf16, name=f"{nm}_sb", tag=f"{nm}_sb")
            nc.sync.dma_start(wt[:], t.rearrange("(kc p) j -> p kc j", p=128))
            return wt

        w_sb = {"wk": w_dma("wk", wk), "wq": w_dma("wq", wq)}

        ctx_mb = [None] * 4
        x_qb = [None] * 4

        def ctx_dma(mb):
            t = dpool.tile([128, KC, 512], bf16, name=f"ctx_{mb}", tag=f"ctx_{mb}")
            nc.sync.dma_start(
                t[:],
                ctxT[:, mb * 512:(mb + 1) * 512].rearrange("(kc p) m -> p kc m", p=128))
            ctx_mb[mb] = t

        def x_dma(qb):
            t = dpool.tile([128, KC, 512], bf16, name=f"x_{qb}", tag=f"x_{qb}")
            nc.sync.dma_start(
                t[:],
                xT[:, qb * 512:(qb + 1) * 512].rearrange("(kc p) m -> p kc m", p=128))
            x_qb[qb] = t

        ctx_dma(0)
        x_dma(0)
        w_sb["wv"] = w_dma("wv", wv)
        for mb in range(1, 4):
            ctx_dma(mb)
        for qb in range(1, 4):
            x_dma(qb)
        wo_sb = []
        for p in range(PAIRS):
            t = wpool.tile([128, D], bf16, name=f"wo_sb{p}", tag=f"wo_sb{p}")
            nc.sync.dma_start(t[:], wo[p * 128:(p + 1) * 128, :])
            wo_sb.append(t)

        ident = wpool.tile([128, 128], bf16, name="ident", tag="ident")
        make_identity(nc, ident[:])

        # PE p-state warmup: keep the tensor engine busy during the input
        # DMA window so the projections run at full clock (ramp > 3us)
        warm = wpool.tile([128, 512], bf16, name="warm", tag="warm")
        nc.vector.memset(warm[:], 0.0)
        for _ in range(28):
            wps = aux_ps.tile([128, 512], f32, name="warm_ps", tag="aux")
            nc.tensor.matmul(wps[:], warm[:, 0:128], warm[:], start=True, stop=True)

        kt_sb = [kqpool.tile([128, M], bf16, name=f"kt_sb{p}", tag=f"kt_sb{p}")
                 for p in range(PAIRS)]
        qt_sb = [kqpool.tile([128, N], bf16, name=f"qt_sb{p}", tag=f"qt_sb{p}")
                 for p in range(PAIRS)]
        v_all = kqpool.tile([128, KT, HL, DH + 1], bf16, name="v_all", tag="v_all")
        ots_sb = [ots_pool.tile([128, N], bf16, name=f"ots_sb{p}", tag=f"ots_sb{p}")
                  for p in range(PAIRS)]

        # ---- projection emitters ----
        def proj_kt(mb, p):
            ps = aux_ps.tile([128, 512], f32, name="kt_ps", tag="aux")
            for kc in range(KC):
                nc.tensor.matmul(ps[:], w_sb["wk"][:, kc, p * 128:(p + 1) * 128],
                                 ctx_mb[mb][:, kc, :],
                                 start=(kc == 0), stop=(kc == KC - 1))
            nc.vector.tensor_copy(kt_sb[p][:, mb * 512:(mb + 1) * 512], ps[:])

        def proj_qt(qb, p):
            ps = aux_ps.tile([128, 512], f32, name="qt_ps", tag="aux")
            for kc in range(KC):
                nc.tensor.matmul(ps[:], w_sb["wq"][:, kc, p * 128:(p + 1) * 128],
                                 x_qb[qb][:, kc, :],
                                 start=(kc == 0), stop=(kc == KC - 1))
            nc.vector.tensor_copy(qt_sb[p][:, qb * 512:(qb + 1) * 512], ps[:])

        def proj_v(kt):
            mb, ms = divmod(kt, 4)
            ps = aux_ps.tile([128, DL], f32, name="v_ps", tag="aux")
            for kc in range(KC):
                nc.tensor.matmul(ps[:], ctx_mb[mb][:, kc, ms * 128:(ms + 1) * 128],
                                 w_sb["wv"][:, kc, :],
                                 start=(kc == 0), stop=(kc == KC - 1))
            nc.vector.memset(v_all[:, kt, :, DH:DH + 1], 1.0)
            nc.vector.tensor_copy(
                v_all[:, kt, :, 0:DH],
                ps.rearrange("p (h j) -> p h j", j=DH))

        # startup-critical: first scores need kt tile 0 and q block 0 (pair 0)
        proj_kt(0, 0)
        proj_qt(0, 0)

        # deadline-ordered filler schedule for the first 512-q block:
        # (p, hh, kp, phase) -> emitters.  phase 0 = before scores (feeds the
        # next scores' K tiles), phase 1 = between exp and AV.
        sched = {}
        for kp in range(8):
            sched[(0, 0, kp, 0)] = [lambda kt=2 * kp: proj_v(kt)]
            sched[(0, 0, kp, 1)] = [lambda kt=2 * kp + 1: proj_v(kt)]
        # kp0: keep the pre-scores hook empty so a late wv DMA cannot
        # head-block the first score matmuls
        sched[(0, 0, 0, 1)] = [lambda: proj_v(0), lambda: proj_v(1)]
        del sched[(0, 0, 0, 0)]
        for kp in (1, 3, 5):
            sched[(0, 0, kp, 1)].append(
                lambda mb=(kp + 1) // 2: proj_kt(mb, 0))
        for kp in range(4):
            sched[(0, 1, 2 * kp, 1)] = [lambda mb=kp: proj_kt(mb, 1)]
        sched[(0, 1, 7, 1)] = [lambda: proj_qt(0, 1)]
        sched[(1, 0, 1, 1)] = [lambda: proj_qt(1, 0)]
        sched[(1, 0, 3, 1)] = [lambda: proj_qt(2, 0)]
        sched[(1, 0, 5, 1)] = [lambda: proj_qt(2, 1)]
        sched[(1, 1, 1, 1)] = [lambda: proj_qt(1, 1)]
        sched[(1, 1, 3, 1)] = [lambda: proj_qt(3, 0)]
        sched[(1, 1, 5, 1)] = [lambda: proj_qt(3, 1)]

        # ---- main attention loop ----
        for qhb in range(NQHB):
            q0 = qhb * QHB
            for p in range(PAIRS):
                ob = ob_pool.tile([128, 4, 128], bf16, name="ob", tag="ob")
                for hh in range(2):
                    hp = slice(hh * 64, hh * 64 + 64)
                    hloc = p * 2 + hh
                    ot = ot_ps.tile([128, 4, DH + 1], f32, name="ot", tag="ot")
                    av_backlog = []

                    def av_emit(pt, kp):
                        for k2 in range(2):
                            kt = 2 * kp + k2
                            for qt in range(4):
                                nc.tensor.matmul(
                                    ot[:, qt, :],
                                    pt[:, k2, qt * 128:(qt + 1) * 128],
                                    v_all[:, kt, hloc, :],
                                    start=(kt == 0 and qt == 0),
                                    stop=(kt == KT - 1 and qt == 3))

                    for kp in range(KT // 2):
                        if qhb == 0:
                            for fn in sched.get((p, hh, kp, 0), ()):
                                fn()
                        st = st_ps.tile([128, 2, QHB], f32, name="st", tag="st")
                        for k2 in range(2):
                            kt = 2 * kp + k2
                            nc.tensor.matmul(
                                st[:, k2, :],
                                kt_sb[p][hp, kt * 128:(kt + 1) * 128],
                                qt_sb[p][hp, q0:q0 + QHB],
                                start=True, stop=True)
                        pt = pt_pool.tile([128, 2, QHB], bf16, name="pt", tag="pt")
                        if qhb > 0 and kp in (2, 4, 6):
                            # offload ~19% of the exps to DVE via the bf16
                            # Schraudolph bitcast approximation (ACT is the
                            # steady-state bottleneck; DVE has headroom)
                            nc.vector.tensor_scalar(
                                out=pt[:].bitcast(i16), in0=st[:],
                                scalar1=EXP_A, scalar2=EXP_B,
                                op0=mybir.AluOpType.mult,
                                op1=mybir.AluOpType.add)
                        else:
                            nc.scalar.activation(pt[:], st[:], EXP)
                        if qhb == 0:
                            for fn in sched.get((p, hh, kp, 1), ()):
                                fn()
                        # lag AV by one k-chunk: projection-copy latencies and
                        # the exp tail hide behind the next chunk's scores
                        av_backlog.append((pt, kp))
                        if len(av_backlog) > 7:
                            av_emit(*av_backlog.pop(0))
                    while av_backlog:
                        av_emit(*av_backlog.pop(0))
                    # normalisation: per-q reciprocal of the denominator col
                    rinv = nrm_pool.tile([128, 4, 1], f32, name="rinv", tag="rinv")
                    nc.vector.reciprocal(rinv[:, :, 0], ot[:, :, DH])
                    nc.vector.tensor_mul(
                        ob[:, :, hh * 64:hh * 64 + 64],
                        ot[:, :, 0:DH],
                        rinv.broadcast_to((128, 4, DH)))
                # transpose O -> O^T for the out-projection; for the last
                # pair interleave per-qt with the out-projection + DMA so the
                # end-of-block tail is a short pipeline
                if p == 0:
                    for qt in range(4):
                        tr = aux_ps.tile([128, 128], bf16, name="tr", tag="aux")
                        nc.tensor.transpose(tr[:], ob[:, qt, :], ident[:])
                        nc.vector.tensor_copy(
                            ots_sb[p][:, q0 + qt * 128:q0 + (qt + 1) * 128], tr[:])
            # out-projection for this 512-q block
            for qt in range(4):
                qq = q0 + qt * 128
                last = qhb == NQHB - 1
                tr = (st_ps if last else aux_ps).tile(
                    [128, 128], bf16, name="tr", tag="st" if last else "aux")
                nc.tensor.transpose(tr[:], ob[:, qt, :], ident[:])
                (nc.scalar.copy if last else nc.vector.tensor_copy)(
                    ots_sb[1][:, qq:qq + 128], tr[:])
                osb = osb_pool.tile([128, D], bf16, name="osb", tag="osb")
                for ec in range(2):
                    ops = (ot_ps if last and ec == 1 else aux_ps).tile(
                        [128, 512], f32, name="o_ps",
                        tag="ot" if last and ec == 1 else "aux")
                    for p in range(PAIRS):
                        nc.tensor.matmul(
                            ops[:],
                            ots_sb[p][:, qq:qq + 128],
                            wo_sb[p][:, ec * 512:(ec + 1) * 512],
                            start=(p == 0), stop=(p == PAIRS - 1))
                    (nc.scalar.copy if last and ec == 1
                     else nc.vector.tensor_copy)(
                        osb[:, ec * 512:(ec + 1) * 512], ops[:])
                nc.sync.dma_start(out[qq:qq + 128, :], osb[:])

    nc.compile()
    return nc


_CACHE = {}


def _get_nc():
    if "nc" not in _CACHE:
        _CACHE["nc"] = build()
    return _CACHE["nc"]


def make_in_maps(x, context, Wq, Wk, Wv, Wo):
    """Shard full inputs into the 8 per-core input dicts (bf16)."""
    x = np.asarray(x, np.float32)
    context = np.asarray(context, np.float32)
    wq_s = (np.asarray(Wq, np.float32) * SCALE).astype(np_bf16)
    wk = np.asarray(Wk, np.float32).astype(np_bf16)
    wv = np.asarray(Wv, np.float32).astype(np_bf16)
    wo = np.asarray(Wo, np.float32).astype(np_bf16)
    xT = [np.ascontiguousarray(x[b].T).astype(np_bf16) for b in range(B)]
    cT = [np.ascontiguousarray(context[b].T).astype(np_bf16) for b in range(B)]
    in_maps = []
    for c in range(NCORES):
        b, g = divmod(c, CPB)
        cols = slice(g * DL, (g + 1) * DL)
        in_maps.append({
            "xT": xT[b],
            "ctxT": cT[b],
            "wq": np.ascontiguousarray(wq_s[:, cols]),
            "wk": np.ascontiguousarray(wk[:, cols]),
            "wv": np.ascontiguousarray(wv[:, cols]),
            "wo": np.ascontiguousarray(wo[cols, :]),
        })
    return in_maps


def combine(partials, bo):
    """Sum per-core partial outputs (Megatron row-parallel all-reduce) + bias."""
    out = np.zeros((B, N, D), np.float32)
    for c in range(NCORES):
        out[c // CPB] += partials[c]
    return out + np.asarray(bo, np.float32)


def kernel(x, context, Wq, Wk, Wv, Wo, bo):
    nc = _get_nc()
    in_maps = make_in_maps(x, context, Wq, Wk, Wv, Wo)
    res = run_bass_kernel_spmd(nc, in_maps, list(range(NCORES)))
    return combine([res.results[c]["out"] for c in range(NCORES)], bo)
